# revision 11
# baseline (speedup 1.0000x reference)
"""Trainium2 Bass kernel for segmented ("sparse") multi-head-ish attention.

Reference math (per batch b of 16, S=1024, H=441):
  q = hs @ Wq + bq ; k = hs @ Wk + bk ; v = hs @ Wv + bv
  for each of 9 column segments [s,e): softmax(q_i k_i^T / sqrt(d_i)) @ v_i
  out = concat(ctx_i) @ Wo + bo

Sharding: pure data parallel over batch, 2 batches per NeuronCore x 8 cores.

Device-side layout strategy (per batch):
  - hs is pre-transposed on host: hsT [441, 1024]   (h on partitions)
  - qT,kT: [441, 1024] computed as W.T-free projections (weights stationary
    = W column slices, moving = hsT), stored in a 32-aligned "packed" layout
    of 5 tiles x 128 partitions so each segment's rows sit at a 32-aligned
    partition base (enables PE row-tiling for the scores matmul).
  - scoresT[t, s] = kT_seg[:, tcol].T @ qT_seg  -> PSUM [128t, 512s]
  - E = exp(scoresT / sqrt(d)) on the scalar engine, PSUM -> SBUF (f32r)
  - v in natural layout [s, 441] with a per-segment ones column appended
    (aug layout, 450 cols): ctx_unnorm^T and the softmax denominator come out
    of one accumulated matmul over the 8 t-chunks.
  - normalize: denominator row gathered by DMA, reciprocal on DVE, broadcast
    across partitions with a K=1 ones matmul, multiply on DVE into packed ctxT.
  - outT[ho, s] = sum over segment row-pieces of Wo_packed.T @ ctxT  (+bo)
  - host transposes outT back to [S, 441].

All matmuls run as float32r (TF32-like, 1 col/cycle at N>=256).
"""

import os
import math
import numpy as np
from contextlib import ExitStack

import concourse.bacc as bacc
import concourse.tile as tile
import concourse.mybir as mybir
from concourse.bass_utils import run_bass_kernel_spmd

F32 = mybir.dt.float32
F32R = mybir.dt.float32r
AF = mybir.ActivationFunctionType

HID = 441
S = 1024
B = 16
N_CORES = 8
BPC = B // N_CORES  # batches per core
BOUNDS = [0, 7, 21, 49, 105, 161, 217, 273, 357, 441]
NSEG = 9
DSEG = [BOUNDS[i + 1] - BOUNDS[i] for i in range(NSEG)]
NHC = 4  # h chunks of 128 (last = 57)
HCH = [(i * 128, min(128, HID - i * 128)) for i in range(NHC)]
NTC = 8  # t chunks of 128
SH = 512  # s half width
NPT = 5  # packed tiles

# Packed row layout pieces: (seg, off_within_seg, length, pack_tile, pack_base)
# All pack bases 32-aligned, K-parts <=64 at {0,64}, <=32 at {0,32,64}.
PIECES = [
    (0, 0, 7, 4, 32),
    (1, 0, 14, 4, 64),
    (2, 0, 28, 4, 0),
    (3, 0, 56, 0, 0),
    (4, 0, 56, 1, 0),
    (5, 0, 56, 0, 64),
    (6, 0, 56, 1, 64),
    (7, 0, 64, 2, 0),
    (7, 64, 20, 3, 0),
    (8, 0, 64, 2, 64),
    (8, 64, 20, 3, 64),
]
SEG_PIECES = [[p for p in PIECES if p[0] == s] for s in range(NSEG)]

# fp32r matmuls accumulating into one PSUM group must all share the same
# lhsT/rhs base partition (mixed-base groups abort at runtime), so the output
# projection accumulates one PSUM bank per base and sums them on DVE.
WO_BASES = [0, 32, 64]
WO_GROUPS = [[p for p in PIECES if p[4] == b] for b in WO_BASES]

# v augmented layout: per segment [v columns (d), ones column]
AUG_OFF = [BOUNDS[i] + i for i in range(NSEG)]  # start col of seg i in aug layout
AUG_W = HID + NSEG  # 450


def _repack_jobs():
    """Split PIECES at 128-row boundaries of the aligned projection chunks.
    Returns [(aligned_chunk, src_row, pack_tile, pack_row, length)]."""
    jobs = []
    for seg, off, length, pt, pb in PIECES:
        g0 = BOUNDS[seg] + off
        done = 0
        while done < length:
            g = g0 + done
            ac = g // 128
            take = min(length - done, (ac + 1) * 128 - g)
            jobs.append((ac, g - ac * 128, pt, pb + done, take))
            done += take
    return jobs


REPACK = _repack_jobs()

_CACHE = {}


def _build():
    stage = int(os.environ.get("KERNEL_STAGE", "9"))
    nc = bacc.Bacc("TRN2", target_bir_lowering=False, debug=False)

    hsT = nc.dram_tensor("hsT", [BPC, HID, S], F32R, kind="ExternalInput").ap()
    Wq_d = nc.dram_tensor("Wq", [HID, HID + 1], F32R, kind="ExternalInput").ap()
    Wk_d = nc.dram_tensor("Wk", [HID, HID + 1], F32R, kind="ExternalInput").ap()
    Wv_d = nc.dram_tensor("Wv", [HID, HID + 1], F32R, kind="ExternalInput").ap()
    Wo_d = nc.dram_tensor("Wop", [NPT, 128, HID + 1], F32R, kind="ExternalInput").ap()
    bq_d = nc.dram_tensor("bq", [HID, 1], F32, kind="ExternalInput").ap()
    bk_d = nc.dram_tensor("bk", [HID, 1], F32, kind="ExternalInput").ap()
    bo_d = nc.dram_tensor("bo", [HID, 1], F32, kind="ExternalInput").ap()
    bvb_d = nc.dram_tensor("bvb", [128, HID], F32, kind="ExternalInput").ap()
    outT = nc.dram_tensor("outT", [BPC, HID, S], F32, kind="ExternalOutput").ap()

    with tile.TileContext(nc) as tc, ExitStack() as ctx, nc.allow_low_precision(
        reason="float32r rounding for matmul inputs"
    ):
        cpool = ctx.enter_context(tc.tile_pool(name="c", bufs=1))
        hpool = ctx.enter_context(tc.tile_pool(name="h", bufs=1))
        apool = ctx.enter_context(tc.tile_pool(name="a", bufs=1))
        ppool = ctx.enter_context(tc.tile_pool(name="p", bufs=1))
        epool = ctx.enter_context(tc.tile_pool(name="e", bufs=2))
        vpool = ctx.enter_context(tc.tile_pool(name="v", bufs=9))
        spool = ctx.enter_context(tc.tile_pool(name="s", bufs=2))
        ps = ctx.enter_context(tc.tile_pool(name="ps", bufs=6, space="PSUM"))
        psv = ctx.enter_context(tc.tile_pool(name="psv", bufs=2, space="PSUM"))

        # ---- constants ----
        Wq_sb, Wk_sb, Wv_sb = [], [], []
        for hc, (h0, hw) in enumerate(HCH):
            for lst, src, nm in ((Wq_sb, Wq_d, "wq"), (Wk_sb, Wk_d, "wk"), (Wv_sb, Wv_d, "wv")):
                t = cpool.tile([hw, HID + 1], F32R, name=f"{nm}{hc}", tag=f"{nm}{hc}")
                nc.sync.dma_start(out=t, in_=src[h0 : h0 + hw, :])
                lst.append(t)
        Wo_sb = []
        for i in range(NPT):
            t = cpool.tile([128, HID + 1], F32R, name=f"wo{i}", tag=f"wo{i}")
            nc.sync.dma_start(out=t, in_=Wo_d[i])
            Wo_sb.append(t)

        bq_sb, bk_sb, bo_sb = [], [], []
        for hc, (h0, hw) in enumerate(HCH):
            for lst, src, nm in ((bq_sb, bq_d, "bq"), (bk_sb, bk_d, "bk"), (bo_sb, bo_d, "bo")):
                t = cpool.tile([hw, 1], F32, name=f"{nm}{hc}", tag=f"{nm}{hc}")
                nc.sync.dma_start(out=t, in_=src[h0 : h0 + hw, :])
                lst.append(t)
        bvb = cpool.tile([128, HID], F32, name="bvb", tag="bvb")
        nc.sync.dma_start(out=bvb, in_=bvb_d)
        ones_c = cpool.tile([1, 128], F32R, name="ones_c", tag="ones")
        nc.vector.memset(ones_c[:].bitcast(F32), 1.0)

        for b in range(BPC):
            # ---- load hsT ----
            hs = []
            for hc, (h0, hw) in enumerate(HCH):
                t = hpool.tile([hw, S], F32R, name=f"hs{hc}", tag=f"hs{hc}", bufs=1)
                nc.sync.dma_start(out=t, in_=hsT[b, h0 : h0 + hw, :])
                hs.append(t)

            # ---- q/k projections into aligned chunks, then DMA-repack ----
            if stage < 1:
                for hc, (h0, hw) in enumerate(HCH):
                    nc.sync.dma_start(out=outT[b, h0 : h0 + hw, :], in_=hs[hc][:].bitcast(F32))
                continue
            packs = {}
            for nm, W_sb, b_sb in (("q", Wq_sb, bq_sb), ("k", Wk_sb, bk_sb)):
                al = []
                for mc, (m0, mw) in enumerate(HCH):
                    qa = apool.tile([mw, S], F32R, name=f"al{nm}{mc}", tag=f"al{mc}", bufs=1)
                    for half in range(2):
                        pa = ps.tile([128, SH], F32, name=f"pp{nm}{mc}{half}", tag="s")
                        for hc, (h0, hw) in enumerate(HCH):
                            nc.tensor.matmul(
                                pa[0:mw, :],
                                Wq_sb[hc][:, m0 : m0 + mw] if nm == "q" else Wk_sb[hc][:, m0 : m0 + mw],
                                hs[hc][:, half * SH : (half + 1) * SH],
                                start=(hc == 0),
                                stop=(hc == NHC - 1),
                            )
                        nc.vector.tensor_scalar_add(
                            qa[:, half * SH : (half + 1) * SH], pa[0:mw, :], b_sb[mc][:]
                        )
                    al.append(qa)
                if stage < 2:
                    continue
                pk = []
                for i in range(NPT):
                    t = ppool.tile([128, S], F32R, name=f"{nm}T{i}", tag=f"{nm}T{i}")
                    pk.append(t)
                for ac, r0, pt, pb, ln in REPACK:
                    nc.gpsimd.dma_start(out=pk[pt][pb : pb + ln, :], in_=al[ac][r0 : r0 + ln, :])
                packs[nm] = pk
            if stage < 2:
                for mc, (m0, mw) in enumerate(HCH):
                    nc.sync.dma_start(out=outT[b, m0 : m0 + mw, :], in_=al[mc][:].bitcast(F32))
                continue
            qT, kT = packs["q"], packs["k"]
            if stage < 3:
                for i in range(NPT - 1):
                    nc.sync.dma_start(out=outT[b, i * 110 : i * 110 + 110, :], in_=qT[i][0:110, :].bitcast(F32))
                continue

            # ---- v projection into natural layout + aug ones columns ----
            vaug = []
            for sc in range(NTC):
                pv = psv.tile([128, HID + 1], F32, name=f"pv{sc}", tag="pv")
                for hc, (h0, hw) in enumerate(HCH):
                    nc.tensor.matmul(
                        pv[:],
                        hs[hc][:, sc * 128 : (sc + 1) * 128],
                        Wv_sb[hc][:],
                        start=(hc == 0),
                        stop=(hc == NHC - 1),
                    )
                va = vpool.tile([128, AUG_W], F32R, name=f"va{sc}", tag="va")
                for sg in range(NSEG):
                    s0, s1 = BOUNDS[sg], BOUNDS[sg + 1]
                    a0 = AUG_OFF[sg]
                    nc.vector.tensor_add(
                        va[:, a0 : a0 + (s1 - s0)], pv[:, s0:s1], bvb[:, s0:s1]
                    )
                    nc.vector.memset(va[:, a0 + (s1 - s0) : a0 + (s1 - s0) + 1].bitcast(F32), 1.0)
                vaug.append(va)

            if stage < 4:
                for sc in range(4):
                    nc.sync.dma_start(out=outT[b, sc * 110 : sc * 110 + 110, :], in_=vaug[sc][:110, 0:S].bitcast(F32))
                continue
            # ---- attention ----
            cxT = [ppool.tile([128, S], F32R, name=f"cxT{i}", tag=f"cxT{i}") for i in range(NPT)]
            for half in range(2):
                hsl = slice(half * SH, (half + 1) * SH)
                for sg in range(NSEG):
                    d = DSEG[sg]
                    scale = 1.0 / math.sqrt(d)
                    pieces = SEG_PIECES[sg]
                    E = epool.tile([128, NTC * SH], F32R, name=f"E{sg}", tag="E")
                    for t in range(NTC):
                        pm = ps.tile([128, SH], F32, name=f"pm{sg}{t}", tag="s")
                        for j, (_, off, ln, pt, pb) in enumerate(pieces):
                            nc.tensor.matmul(
                                pm[:],
                                kT[pt][pb : pb + ln, t * 128 : (t + 1) * 128],
                                qT[pt][pb : pb + ln, hsl],
                                start=(j == 0),
                                stop=(j == len(pieces) - 1),
                            )
                        nc.scalar.activation(
                            E[:, t * SH : (t + 1) * SH], pm[:], AF.Exp, scale=scale
                        )
                    # ctx_unnorm^T + denominator via aug ones column
                    pu = ps.tile([128, SH], F32, name=f"pu{sg}", tag="s")
                    a0 = AUG_OFF[sg]
                    for t in range(NTC):
                        nc.tensor.matmul(
                            pu[0 : d + 1, :],
                            vaug[t][:, a0 : a0 + d + 1],
                            E[:, t * SH : (t + 1) * SH],
                            start=(t == 0),
                            stop=(t == NTC - 1),
                        )
                    if stage < 5:
                        if sg == 0:
                            nc.sync.dma_start(out=outT[b, 0:128, hsl], in_=E[:, 0:SH].bitcast(F32))
                        continue
                    u = spool.tile([96, SH], F32, name=f"u{sg}", tag="u")
                    nc.vector.tensor_copy(u[0 : d + 1, :], pu[0 : d + 1, :])
                    den = spool.tile([1, SH], F32, name=f"den{sg}", tag="den", bufs=1)
                    nc.gpsimd.dma_start(out=den[:], in_=u[d : d + 1, :])
                    rec = spool.tile([1, SH], F32R, name=f"rec{sg}", tag="rec", bufs=1)
                    nc.vector.reciprocal(rec[:], den[:])
                    pb_ps = ps.tile([128, SH], F32, name=f"pb{sg}", tag="s")
                    nc.tensor.matmul(pb_ps[0:d, :], ones_c[:, 0:d], rec[:], start=True, stop=True)
                    bc = spool.tile([96, SH], F32, name=f"bc{sg}", tag="bc")
                    nc.vector.tensor_copy(bc[0:d, :], pb_ps[0:d, :])
                    for _, off, ln, pt, pbs in pieces:
                        nc.vector.tensor_mul(
                            cxT[pt][pbs : pbs + ln, hsl],
                            u[off : off + ln, :],
                            bc[off : off + ln, :],
                        )

                if stage < 6:
                    for i in range(NPT - 1):
                        nc.sync.dma_start(out=outT[b, i * 110 : i * 110 + 110, hsl], in_=cxT[i][0:110, hsl].bitcast(F32))
                    continue
                # ---- output projection for this half ----
                _wo_sel = os.environ.get("KERNEL_WO_HC")
                for hc, (h0, hw) in enumerate(HCH):
                    if _wo_sel is not None and hc not in [int(x) for x in _wo_sel.split(",")]:
                        continue
                    pos = []
                    for g, plist in enumerate(WO_GROUPS):
                        po = ps.tile([128, SH], F32, name=f"po{hc}{g}", tag="s")
                        for j, (_, off, ln, pt, pbs) in enumerate(plist):
                            nc.tensor.matmul(
                                po[0:hw, :],
                                Wo_sb[pt][pbs : pbs + ln, h0 : h0 + hw],
                                cxT[pt][pbs : pbs + ln, hsl],
                                start=(j == 0),
                                stop=(j == len(plist) - 1),
                            )
                        pos.append(po)
                    osb = spool.tile([128, SH], F32, name=f"osb{hc}", tag="osb", bufs=1)
                    nc.vector.tensor_scalar_add(osb[0:hw, :], pos[0][0:hw, :], bo_sb[hc][:])
                    for po in pos[1:]:
                        nc.vector.tensor_add(osb[0:hw, :], osb[0:hw, :], po[0:hw, :])
                    nc.sync.dma_start(
                        out=outT[b, h0 : h0 + hw, hsl], in_=osb[0:hw, :]
                    )

    nc.compile()
    return nc


def _prep_core_inputs(hidden_states, Wq, bq, Wk, bk, Wv, bv, Wo, bo):
    """Host-side layout prep (transpose/reorder only, no math)."""
    f32 = np.float32
    hs = np.ascontiguousarray(hidden_states.astype(f32, copy=False))
    Wo_p = np.zeros((NPT, 128, HID + 1), dtype=f32)
    for seg, off, ln, pt, pbs in PIECES:
        g0 = BOUNDS[seg] + off
        Wo_p[pt, pbs : pbs + ln, :HID] = Wo[g0 : g0 + ln, :]
    bvb = np.broadcast_to(bv.astype(f32, copy=False), (128, HID)).copy()
    shared = {
        "Wq": np.ascontiguousarray(
            np.concatenate([Wq.astype(f32, copy=False), np.zeros((HID, 1), f32)], axis=1)
        ),
        "Wk": np.ascontiguousarray(
            np.concatenate([Wk.astype(f32, copy=False), np.zeros((HID, 1), f32)], axis=1)
        ),
        "Wv": np.ascontiguousarray(
            np.concatenate([Wv.astype(f32, copy=False), np.zeros((HID, 1), f32)], axis=1)
        ),
        "Wop": Wo_p,
        "bq": np.ascontiguousarray(bq.astype(f32, copy=False).reshape(HID, 1)),
        "bk": np.ascontiguousarray(bk.astype(f32, copy=False).reshape(HID, 1)),
        "bo": np.ascontiguousarray(bo.astype(f32, copy=False).reshape(HID, 1)),
        "bvb": bvb,
    }
    in_maps = []
    for c in range(N_CORES):
        shard = hs[c * BPC : (c + 1) * BPC]  # [BPC, S, HID]
        hsT = np.ascontiguousarray(shard.transpose(0, 2, 1))  # [BPC, HID, S]
        m = dict(shared)
        m["hsT"] = hsT
        in_maps.append(m)
    return in_maps


LAST_RESULTS = None


def kernel(hidden_states, Wq, bq, Wk, bk, Wv, bv, Wo, bo):
    global LAST_RESULTS
    if "nc" not in _CACHE:
        _CACHE["nc"] = _build()
    nc = _CACHE["nc"]
    in_maps = _prep_core_inputs(hidden_states, Wq, bq, Wk, bk, Wv, bv, Wo, bo)
    trace = os.environ.get("KERNEL_TRACE") == "1"
    kwargs = {}
    if trace:
        kwargs["trace"] = True
        td = os.environ.get("KERNEL_TRACE_DIR")
        if td:
            kwargs["tmpdir"] = td
    res = run_bass_kernel_spmd(nc, in_maps, core_ids=list(range(N_CORES)), **kwargs)
    LAST_RESULTS = res
    out = np.empty((B, S, HID), dtype=np.float32)
    for c in range(N_CORES):
        outT = res.results[c]["outT"]  # [BPC, HID, S]
        out[c * BPC : (c + 1) * BPC] = outT.transpose(0, 2, 1)
    return out


# revision 14
# speedup vs baseline: 1.4256x; 1.4256x over previous
"""Trainium2 Bass kernel for segmented ("sparse") attention.

Reference math (per batch of 16, S=1024, H=441):
  q = hs @ Wq + bq ; k = hs @ Wk + bk ; v = hs @ Wv + bv
  for each of 9 column segments [s,e): softmax(q_i k_i^T / sqrt(d_i)) @ v_i
  out = concat(ctx_i) @ Wo + bo

Sharding: pure data parallel over batch, 2 batches per NeuronCore x 8 cores.

Device-side strategy (per batch):
  - hs pre-transposed on host: hsT [441, 1024] (h on partitions)
  - qT,kT [441, 1024] projections stored in a packed layout of 5 tiles where
    each segment's rows sit at a 32-aligned partition base
  - scoresT[t, s] = kT_seg[:, tcol].T @ qT_seg -> PSUM [128t, 512s]
  - E = exp(scoresT / sqrt(d)) on the scalar engine, PSUM -> SBUF
  - v in natural layout [s, 441] with a ones column appended per segment
    (aug layout): one accumulated matmul over the 8 t-chunks yields both
    ctx_unnorm^T and the softmax denominator
  - normalize: denom row DMA-gathered, reciprocal on DVE, broadcast across
    partitions with a K=1 ones matmul, multiplied into packed ctxT
  - outT[ho, s] = Wo_packed.T @ ctxT (+bo), accumulated per base-partition
    group (fp32r groups must not mix lhsT base partitions) and summed on DVE
  - host transposes outT back to [S, 441]

All matmuls are float32r (TF32-like). fp32r oddities honored: moving/dst
free sizes even, out base partition 0, uniform base per accumulation group.
"""

import os
import math
import numpy as np
from contextlib import ExitStack

import concourse.bacc as bacc
import concourse.tile as tile
import concourse.mybir as mybir
from concourse.bass_utils import run_bass_kernel_spmd

F32 = mybir.dt.float32
F32R = mybir.dt.float32r
AF = mybir.ActivationFunctionType

HID = 441
HID2 = HID + 1  # even-padded weight width
S = 1024
B = 16
N_CORES = 8
BPC = B // N_CORES
BOUNDS = [0, 7, 21, 49, 105, 161, 217, 273, 357, 441]
NSEG = 9
DSEG = [BOUNDS[i + 1] - BOUNDS[i] for i in range(NSEG)]
NHC = 4
HCH = [(i * 128, min(128, HID - i * 128)) for i in range(NHC)]
NTC = 8
SH = 512
NPT = 5

# Packed row layout: (seg, off_within_seg, length, pack_tile, pack_base)
# One piece per segment; bases 32-aligned; base 96 passed as explicit
# tile_position (auto-derive only allows 0/32/64).
PIECES = [
    (0, 0, 7, 4, 0),
    (1, 0, 14, 1, 96),
    (2, 0, 28, 0, 96),
    (3, 0, 56, 2, 0),
    (4, 0, 56, 3, 0),
    (5, 0, 56, 2, 64),
    (6, 0, 56, 3, 64),
    (7, 0, 84, 0, 0),
    (8, 0, 84, 1, 0),
]
SEG_PIECE = {p[0]: p for p in PIECES}

# output projection: one PSUM accumulation group per lhsT base partition
WO_GROUPS = [[p for p in PIECES if p[4] == b] for b in (0, 64, 96)]

# v augmented layout: per segment [v columns (d), ones column]
AUG_OFF = [BOUNDS[i] + i for i in range(NSEG)]
AUG_W = HID + NSEG  # 450


def _repack_jobs():
    jobs = []
    for seg, off, length, pt, pb in PIECES:
        g0 = BOUNDS[seg] + off
        done = 0
        while done < length:
            g = g0 + done
            ac = g // 128
            take = min(length - done, (ac + 1) * 128 - g)
            jobs.append((ac, g - ac * 128, pt, pb + done, take))
            done += take
    return jobs


REPACK = _repack_jobs()

_CACHE = {}


def _build():
    nc = bacc.Bacc("TRN2", target_bir_lowering=False, debug=False)

    hsT = nc.dram_tensor("hsT", [BPC, HID, S], F32R, kind="ExternalInput").ap()
    Wq_d = nc.dram_tensor("Wq", [HID, HID2], F32R, kind="ExternalInput").ap()
    Wk_d = nc.dram_tensor("Wk", [HID, HID2], F32R, kind="ExternalInput").ap()
    Wv_d = nc.dram_tensor("Wv", [HID, HID2], F32R, kind="ExternalInput").ap()
    Wo_d = nc.dram_tensor("Wop", [NPT, 128, HID2], F32R, kind="ExternalInput").ap()
    bq_d = nc.dram_tensor("bq", [HID, 1], F32, kind="ExternalInput").ap()
    bk_d = nc.dram_tensor("bk", [HID, 1], F32, kind="ExternalInput").ap()
    bo_d = nc.dram_tensor("bo", [HID, 1], F32, kind="ExternalInput").ap()
    bvb_d = nc.dram_tensor("bvb", [128, HID], F32, kind="ExternalInput").ap()
    outT = nc.dram_tensor("outT", [BPC, HID, S], F32, kind="ExternalOutput").ap()

    with tile.TileContext(nc) as tc, ExitStack() as ctx, nc.allow_low_precision(
        reason="float32r rounding for matmul inputs"
    ):
        cpool = ctx.enter_context(tc.tile_pool(name="c", bufs=1))
        hpool = ctx.enter_context(tc.tile_pool(name="h", bufs=1))
        apool = ctx.enter_context(tc.tile_pool(name="a", bufs=1))
        ppool = ctx.enter_context(tc.tile_pool(name="p", bufs=1))
        epool = ctx.enter_context(tc.tile_pool(name="e", bufs=2))
        vpool = ctx.enter_context(tc.tile_pool(name="v", bufs=9))
        spool = ctx.enter_context(tc.tile_pool(name="s", bufs=2))
        ps = ctx.enter_context(tc.tile_pool(name="ps", bufs=1, space="PSUM"))

        # ---- constants ----
        Wq_sb, Wk_sb, Wv_sb = [], [], []
        for hc, (h0, hw) in enumerate(HCH):
            for lst, src, nm in ((Wq_sb, Wq_d, "wq"), (Wk_sb, Wk_d, "wk"), (Wv_sb, Wv_d, "wv")):
                t = cpool.tile([hw, HID2], F32R, name=f"{nm}{hc}", tag=f"{nm}{hc}")
                nc.sync.dma_start(out=t, in_=src[h0 : h0 + hw, :])
                lst.append(t)
        Wo_sb = []
        for i in range(NPT):
            t = cpool.tile([128, HID2], F32R, name=f"wo{i}", tag=f"wo{i}")
            nc.sync.dma_start(out=t, in_=Wo_d[i])
            Wo_sb.append(t)
        bq_sb, bk_sb, bo_sb = [], [], []
        for hc, (h0, hw) in enumerate(HCH):
            for lst, src, nm in ((bq_sb, bq_d, "bq"), (bk_sb, bk_d, "bk"), (bo_sb, bo_d, "bo")):
                t = cpool.tile([hw, 1], F32, name=f"{nm}{hc}", tag=f"{nm}{hc}")
                nc.sync.dma_start(out=t, in_=src[h0 : h0 + hw, :])
                lst.append(t)
        bvb = cpool.tile([128, HID], F32, name="bvb", tag="bvb")
        nc.sync.dma_start(out=bvb, in_=bvb_d)
        ones_c = cpool.tile([1, 128], F32R, name="ones_c", tag="ones")
        nc.vector.memset(ones_c[:].bitcast(F32), 1.0)

        for b in range(BPC):
            # ---- load hsT ----
            hs = []
            for hc, (h0, hw) in enumerate(HCH):
                t = hpool.tile([hw, S], F32R, name=f"hs{hc}", tag=f"hs{hc}", bufs=1)
                nc.sync.dma_start(out=t, in_=hsT[b, h0 : h0 + hw, :])
                hs.append(t)

            # ---- q/k projections into aligned chunks, DMA-repacked ----
            packs = {}
            for nm, W_sb, b_sb in (("q", Wq_sb, bq_sb), ("k", Wk_sb, bk_sb)):
                al = []
                for mc, (m0, mw) in enumerate(HCH):
                    qa = apool.tile([mw, S], F32R, name=f"al{nm}{mc}", tag=f"al{mc}", bufs=1)
                    for half in range(2):
                        pa = ps.tile([128, SH], F32, name=f"pp{nm}{mc}{half}", tag="x", bufs=2)
                        for hc, (h0, hw) in enumerate(HCH):
                            nc.tensor.matmul(
                                pa[0:mw, :],
                                W_sb[hc][:, m0 : m0 + mw],
                                hs[hc][:, half * SH : (half + 1) * SH],
                                start=(hc == 0),
                                stop=(hc == NHC - 1),
                            )
                        nc.vector.tensor_scalar_add(
                            qa[:, half * SH : (half + 1) * SH], pa[0:mw, :], b_sb[mc][:]
                        )
                    al.append(qa)
                pk = []
                for i in range(NPT):
                    t = ppool.tile([128, S], F32R, name=f"{nm}T{i}", tag=f"{nm}T{i}")
                    pk.append(t)
                for ac, r0, pt, pb, ln in REPACK:
                    nc.gpsimd.dma_start(out=pk[pt][pb : pb + ln, :], in_=al[ac][r0 : r0 + ln, :])
                packs[nm] = pk
            qT, kT = packs["q"], packs["k"]

            # ---- v projection (natural layout) + aug ones columns ----
            vaug = []
            for sc in range(NTC):
                pv = ps.tile([128, HID2], F32, name=f"pv{sc}", tag="x", bufs=2)
                for hc, (h0, hw) in enumerate(HCH):
                    nc.tensor.matmul(
                        pv[:],
                        hs[hc][:, sc * 128 : (sc + 1) * 128],
                        Wv_sb[hc][:],
                        start=(hc == 0),
                        stop=(hc == NHC - 1),
                    )
                va = vpool.tile([128, AUG_W], F32R, name=f"va{sc}", tag="va")
                for sg in range(NSEG):
                    s0, s1 = BOUNDS[sg], BOUNDS[sg + 1]
                    a0 = AUG_OFF[sg]
                    nc.vector.tensor_add(
                        va[:, a0 : a0 + (s1 - s0)], pv[:, s0:s1], bvb[:, s0:s1]
                    )
                    nc.vector.memset(va[:, a0 + (s1 - s0) : a0 + (s1 - s0) + 1].bitcast(F32), 1.0)
                vaug.append(va)

            # ---- attention ----
            cxT = [ppool.tile([128, S], F32R, name=f"cxT{i}", tag=f"cxT{i}") for i in range(NPT)]
            for half in range(2):
                hsl = slice(half * SH, (half + 1) * SH)
                for sg in range(NSEG):
                    d = DSEG[sg]
                    scale = 1.0 / math.sqrt(d)
                    _, off, ln, pt, pb = SEG_PIECE[sg]
                    tp = (96, 0) if pb == 96 else None
                    E = epool.tile([128, NTC * SH], F32R, name=f"E{sg}", tag="E")
                    for t in range(NTC):
                        pm = ps.tile([128, SH], F32, name=f"pm{sg}{t}", tag="m1", bufs=3)
                        nc.tensor.matmul(
                            pm[:],
                            kT[pt][pb : pb + ln, t * 128 : (t + 1) * 128],
                            qT[pt][pb : pb + ln, hsl],
                            start=True,
                            stop=True,
                            tile_position=tp,
                        )
                        nc.scalar.activation(
                            E[:, t * SH : (t + 1) * SH], pm[:], AF.Exp, scale=scale
                        )
                    # ctx_unnorm^T and denominator via aug ones column
                    pu = ps.tile([128, SH], F32, name=f"pu{sg}", tag="m2", bufs=2)
                    a0 = AUG_OFF[sg]
                    for t in range(NTC):
                        nc.tensor.matmul(
                            pu[0 : d + 1, :],
                            vaug[t][:, a0 : a0 + d + 1],
                            E[:, t * SH : (t + 1) * SH],
                            start=(t == 0),
                            stop=(t == NTC - 1),
                        )
                    u = spool.tile([96, SH], F32, name=f"u{sg}", tag="u")
                    nc.vector.tensor_copy(u[0 : d + 1, :], pu[0 : d + 1, :])
                    den = spool.tile([1, SH], F32, name=f"den{sg}", tag="den", bufs=1)
                    nc.gpsimd.dma_start(out=den[:], in_=u[d : d + 1, :])
                    rec = spool.tile([1, SH], F32, name=f"rec{sg}", tag="rec", bufs=1)
                    scr = spool.tile([1, SH], F32, name=f"scr{sg}", tag="scr", bufs=1)
                    nc.vector.reciprocal_approx_accurate(rec[:], den[:], scratch=scr[:])
                    recr = spool.tile([1, SH], F32R, name=f"recr{sg}", tag="recr", bufs=1)
                    nc.vector.tensor_copy(recr[:], rec[:])
                    pb_ps = ps.tile([128, SH], F32, name=f"pb{sg}", tag="x", bufs=2)
                    nc.tensor.matmul(pb_ps[0:d, :], ones_c[:, 0:d], recr[:], start=True, stop=True)
                    bc = spool.tile([96, SH], F32, name=f"bc{sg}", tag="bc")
                    nc.vector.tensor_copy(bc[0:d, :], pb_ps[0:d, :])
                    nc.vector.tensor_mul(
                        cxT[pt][pb : pb + d, hsl], u[0:d, :], bc[0:d, :]
                    )

                # ---- output projection for this half ----
                for hc, (h0, hw) in enumerate(HCH):
                    pos = []
                    for g, plist in enumerate(WO_GROUPS):
                        po = ps.tile([128, SH], F32, name=f"po{hc}{g}", tag="m1", bufs=3)
                        for j, (_, off, ln, pt, pb) in enumerate(plist):
                            nc.tensor.matmul(
                                po[0:hw, :],
                                Wo_sb[pt][pb : pb + ln, h0 : h0 + hw],
                                cxT[pt][pb : pb + ln, hsl],
                                start=(j == 0),
                                stop=(j == len(plist) - 1),
                                tile_position=(96, 0) if pb == 96 else None,
                            )
                        pos.append(po)
                    osb = spool.tile([128, SH], F32, name=f"osb{hc}", tag="osb", bufs=2)
                    nc.vector.tensor_scalar_add(osb[0:hw, :], pos[0][0:hw, :], bo_sb[hc][:])
                    for po in pos[1:]:
                        nc.vector.tensor_add(osb[0:hw, :], osb[0:hw, :], po[0:hw, :])
                    nc.sync.dma_start(out=outT[b, h0 : h0 + hw, hsl], in_=osb[0:hw, :])

    nc.compile()
    return nc


def _pad_w(W):
    f32 = np.float32
    return np.ascontiguousarray(
        np.concatenate([W.astype(f32, copy=False), np.zeros((HID, 1), f32)], axis=1)
    )


def _prep_core_inputs(hidden_states, Wq, bq, Wk, bk, Wv, bv, Wo, bo):
    """Host-side layout prep (transpose/reorder/pad only, no math)."""
    f32 = np.float32
    hs = np.ascontiguousarray(hidden_states.astype(f32, copy=False))
    Wo_p = np.zeros((NPT, 128, HID2), dtype=f32)
    for seg, off, ln, pt, pb in PIECES:
        g0 = BOUNDS[seg] + off
        Wo_p[pt, pb : pb + ln, :HID] = Wo[g0 : g0 + ln, :]
    bvb = np.broadcast_to(bv.astype(f32, copy=False), (128, HID)).copy()
    shared = {
        "Wq": _pad_w(Wq),
        "Wk": _pad_w(Wk),
        "Wv": _pad_w(Wv),
        "Wop": Wo_p,
        "bq": np.ascontiguousarray(bq.astype(f32, copy=False).reshape(HID, 1)),
        "bk": np.ascontiguousarray(bk.astype(f32, copy=False).reshape(HID, 1)),
        "bo": np.ascontiguousarray(bo.astype(f32, copy=False).reshape(HID, 1)),
        "bvb": bvb,
    }
    in_maps = []
    for c in range(N_CORES):
        shard = hs[c * BPC : (c + 1) * BPC]
        m = dict(shared)
        m["hsT"] = np.ascontiguousarray(shard.transpose(0, 2, 1))
        in_maps.append(m)
    return in_maps


LAST_RESULTS = None


def kernel(hidden_states, Wq, bq, Wk, bk, Wv, bv, Wo, bo):
    global LAST_RESULTS
    if "nc" not in _CACHE:
        _CACHE["nc"] = _build()
    nc = _CACHE["nc"]
    in_maps = _prep_core_inputs(hidden_states, Wq, bq, Wk, bk, Wv, bv, Wo, bo)
    kwargs = {}
    if os.environ.get("KERNEL_TRACE") == "1":
        kwargs["trace"] = True
        td = os.environ.get("KERNEL_TRACE_DIR")
        if td:
            kwargs["tmpdir"] = td
    res = run_bass_kernel_spmd(nc, in_maps, core_ids=list(range(N_CORES)), **kwargs)
    LAST_RESULTS = res
    out = np.empty((B, S, HID), dtype=np.float32)
    for c in range(N_CORES):
        out[c * BPC : (c + 1) * BPC] = res.results[c]["outT"].transpose(0, 2, 1)
    return out


# revision 16
# speedup vs baseline: 1.5113x; 1.0601x over previous
"""Trainium2 Bass kernel for segmented ("sparse") attention.

Reference math (per batch of 16, S=1024, H=441):
  q = hs @ Wq + bq ; k = hs @ Wk + bk ; v = hs @ Wv + bv
  for each of 9 column segments [s,e): softmax(q_i k_i^T / sqrt(d_i)) @ v_i
  out = concat(ctx_i) @ Wo + bo

Sharding: pure data parallel over batch, 2 batches per NeuronCore x 8 cores.

Device-side strategy (per batch):
  - hs pre-transposed on host: hsT [441, 1024] (h on partitions)
  - qT,kT [441, 1024] projections stored in a packed layout of 5 tiles where
    each segment's rows sit at a 32-aligned partition base
  - scoresT[t, s] = kT_seg[:, tcol].T @ qT_seg -> PSUM [128t, 512s]
  - E = exp(scoresT / sqrt(d)) on the scalar engine, PSUM -> SBUF
  - v in natural layout [s, 441] with a ones column appended per segment
    (aug layout): one accumulated matmul over the 8 t-chunks yields both
    ctx_unnorm^T and the softmax denominator
  - normalize: denom row DMA-gathered, reciprocal on DVE, broadcast across
    partitions with a K=1 ones matmul, multiplied into packed ctxT
  - outT[ho, s] = Wo_packed.T @ ctxT (+bo), accumulated per base-partition
    group (fp32r groups must not mix lhsT base partitions) and summed on DVE
  - host transposes outT back to [S, 441]

All matmuls are float32r (TF32-like). fp32r oddities honored: moving/dst
free sizes even, out base partition 0, uniform base per accumulation group.
"""

import os
import math
import numpy as np
from contextlib import ExitStack

import concourse.bacc as bacc
import concourse.tile as tile
import concourse.mybir as mybir
from concourse.bass_utils import run_bass_kernel_spmd

F32 = mybir.dt.float32
F32R = mybir.dt.float32r
AF = mybir.ActivationFunctionType

HID = 441
HID2 = HID + 1  # even-padded weight width
S = 1024
B = 16
N_CORES = 8
BPC = B // N_CORES
BOUNDS = [0, 7, 21, 49, 105, 161, 217, 273, 357, 441]
NSEG = 9
DSEG = [BOUNDS[i + 1] - BOUNDS[i] for i in range(NSEG)]
NHC = 4
HCH = [(i * 128, min(128, HID - i * 128)) for i in range(NHC)]
NTC = 8
SH = 512
NPT = 5

# Packed row layout: (seg, off_within_seg, length, pack_tile, pack_base)
# One piece per segment; bases 32-aligned; base 96 passed as explicit
# tile_position (auto-derive only allows 0/32/64).
PIECES = [
    (0, 0, 7, 4, 0),
    (1, 0, 14, 1, 96),
    (2, 0, 28, 0, 96),
    (3, 0, 56, 2, 0),
    (4, 0, 56, 3, 0),
    (5, 0, 56, 2, 64),
    (6, 0, 56, 3, 64),
    (7, 0, 84, 0, 0),
    (8, 0, 84, 1, 0),
]
SEG_PIECE = {p[0]: p for p in PIECES}

# output projection: one PSUM accumulation group per lhsT base partition
WO_GROUPS = [[p for p in PIECES if p[4] == b] for b in (0, 64, 96)]

# v augmented layout: per segment [v columns (d), ones column]
AUG_OFF = [BOUNDS[i] + i for i in range(NSEG)]
AUG_W = HID + NSEG  # 450


def _repack_jobs():
    jobs = []
    for seg, off, length, pt, pb in PIECES:
        g0 = BOUNDS[seg] + off
        done = 0
        while done < length:
            g = g0 + done
            ac = g // 128
            take = min(length - done, (ac + 1) * 128 - g)
            jobs.append((ac, g - ac * 128, pt, pb + done, take))
            done += take
    return jobs


REPACK = _repack_jobs()

_CACHE = {}


def _build():
    nc = bacc.Bacc("TRN2", target_bir_lowering=False, debug=False)

    hsT = nc.dram_tensor("hsT", [BPC, HID, S], F32R, kind="ExternalInput").ap()
    Wq_d = nc.dram_tensor("Wq", [HID, HID2], F32R, kind="ExternalInput").ap()
    Wk_d = nc.dram_tensor("Wk", [HID, HID2], F32R, kind="ExternalInput").ap()
    Wv_d = nc.dram_tensor("Wv", [HID, HID2], F32R, kind="ExternalInput").ap()
    Wo_d = nc.dram_tensor("Wop", [NPT, 128, HID2], F32R, kind="ExternalInput").ap()
    bq_d = nc.dram_tensor("bq", [HID, 1], F32, kind="ExternalInput").ap()
    bk_d = nc.dram_tensor("bk", [HID, 1], F32, kind="ExternalInput").ap()
    bo_d = nc.dram_tensor("bo", [HID, 1], F32, kind="ExternalInput").ap()
    bvb_d = nc.dram_tensor("bvb", [128, HID], F32, kind="ExternalInput").ap()
    outT = nc.dram_tensor("outT", [BPC, HID, S], F32, kind="ExternalOutput").ap()

    with tile.TileContext(nc) as tc, ExitStack() as ctx, nc.allow_low_precision(
        reason="float32r rounding for matmul inputs"
    ):
        cpool = ctx.enter_context(tc.tile_pool(name="c", bufs=1))
        hpool = ctx.enter_context(tc.tile_pool(name="h", bufs=1))
        apool = ctx.enter_context(tc.tile_pool(name="a", bufs=1))
        ppool = ctx.enter_context(tc.tile_pool(name="p", bufs=1))
        epool = ctx.enter_context(tc.tile_pool(name="e", bufs=2))
        vpool = ctx.enter_context(tc.tile_pool(name="v", bufs=9))
        spool = ctx.enter_context(tc.tile_pool(name="s", bufs=2))
        ps = ctx.enter_context(tc.tile_pool(name="ps", bufs=1, space="PSUM"))

        # ---- constants ----
        Wq_sb, Wk_sb, Wv_sb = [], [], []
        for hc, (h0, hw) in enumerate(HCH):
            for lst, src, nm in ((Wq_sb, Wq_d, "wq"), (Wk_sb, Wk_d, "wk"), (Wv_sb, Wv_d, "wv")):
                t = cpool.tile([hw, HID2], F32R, name=f"{nm}{hc}", tag=f"{nm}{hc}")
                nc.sync.dma_start(out=t, in_=src[h0 : h0 + hw, :])
                lst.append(t)
        Wo_sb = []
        for i in range(NPT):
            t = cpool.tile([128, HID2], F32R, name=f"wo{i}", tag=f"wo{i}")
            nc.sync.dma_start(out=t, in_=Wo_d[i])
            Wo_sb.append(t)
        bq_sb, bk_sb, bo_sb = [], [], []
        for hc, (h0, hw) in enumerate(HCH):
            for lst, src, nm in ((bq_sb, bq_d, "bq"), (bk_sb, bk_d, "bk"), (bo_sb, bo_d, "bo")):
                t = cpool.tile([hw, 1], F32, name=f"{nm}{hc}", tag=f"{nm}{hc}")
                nc.sync.dma_start(out=t, in_=src[h0 : h0 + hw, :])
                lst.append(t)
        bvb = cpool.tile([128, HID], F32, name="bvb", tag="bvb")
        nc.sync.dma_start(out=bvb, in_=bvb_d)
        ones_c = cpool.tile([1, 128], F32R, name="ones_c", tag="ones")
        nc.vector.memset(ones_c[:].bitcast(F32), 1.0)

        for b in range(BPC):
            # ---- load hsT ----
            hs = []
            for hc, (h0, hw) in enumerate(HCH):
                t = hpool.tile([hw, S], F32R, name=f"hs{hc}", tag=f"hs{hc}", bufs=1)
                nc.sync.dma_start(out=t, in_=hsT[b, h0 : h0 + hw, :])
                hs.append(t)

            # ---- q/k projections into aligned chunks, DMA-repacked ----
            packs = {}
            for nm, W_sb, b_sb in (("q", Wq_sb, bq_sb), ("k", Wk_sb, bk_sb)):
                al = []
                for mc, (m0, mw) in enumerate(HCH):
                    qa = apool.tile([mw, S], F32R, name=f"al{nm}{mc}", tag=f"al{mc}", bufs=1)
                    for half in range(2):
                        pa = ps.tile([128, SH], F32, name=f"pp{nm}{mc}{half}", tag="x", bufs=2)
                        for hc, (h0, hw) in enumerate(HCH):
                            nc.tensor.matmul(
                                pa[0:mw, :],
                                W_sb[hc][:, m0 : m0 + mw],
                                hs[hc][:, half * SH : (half + 1) * SH],
                                start=(hc == 0),
                                stop=(hc == NHC - 1),
                            )
                        nc.vector.tensor_scalar_add(
                            qa[:, half * SH : (half + 1) * SH], pa[0:mw, :], b_sb[mc][:]
                        )
                    al.append(qa)
                pk = []
                for i in range(NPT):
                    t = ppool.tile([128, S], F32R, name=f"{nm}T{i}", tag=f"{nm}T{i}")
                    pk.append(t)
                for ac, r0, pt, pb, ln in REPACK:
                    nc.gpsimd.dma_start(out=pk[pt][pb : pb + ln, :], in_=al[ac][r0 : r0 + ln, :])
                packs[nm] = pk
            qT, kT = packs["q"], packs["k"]

            # ---- v projection (natural layout) + aug ones columns ----
            vaug = []
            for sc in range(NTC):
                pv = ps.tile([128, HID2], F32, name=f"pv{sc}", tag="x", bufs=2)
                for hc, (h0, hw) in enumerate(HCH):
                    nc.tensor.matmul(
                        pv[:],
                        hs[hc][:, sc * 128 : (sc + 1) * 128],
                        Wv_sb[hc][:],
                        start=(hc == 0),
                        stop=(hc == NHC - 1),
                    )
                vt = spool.tile([128, HID], F32R, name=f"vt{sc}", tag="vt")
                nc.vector.tensor_add(vt[:], pv[:, 0:HID], bvb[:])
                va = vpool.tile([128, AUG_W], F32R, name=f"va{sc}", tag="va")
                for sg in range(NSEG):
                    s0, s1 = BOUNDS[sg], BOUNDS[sg + 1]
                    a0 = AUG_OFF[sg]
                    nc.gpsimd.dma_start(out=va[:, a0 : a0 + (s1 - s0)], in_=vt[:, s0:s1])
                    nc.vector.memset(va[:, a0 + (s1 - s0) : a0 + (s1 - s0) + 1].bitcast(F32), 1.0)
                vaug.append(va)

            # ---- attention ----
            cxT = [ppool.tile([128, S], F32R, name=f"cxT{i}", tag=f"cxT{i}") for i in range(NPT)]
            for half in range(2):
                hsl = slice(half * SH, (half + 1) * SH)
                for sg in range(NSEG):
                    d = DSEG[sg]
                    scale = 1.0 / math.sqrt(d)
                    _, off, ln, pt, pb = SEG_PIECE[sg]
                    tp = (96, 0) if pb == 96 else None
                    E = epool.tile([128, NTC * SH], F32R, name=f"E{sg}", tag="E")
                    for t2 in range(NTC // 2):
                        pm = ps.tile([128, 2 * SH], F32, name=f"pm{sg}{t2}", tag="m1", bufs=2)
                        for k2 in range(2):
                            t = 2 * t2 + k2
                            nc.tensor.matmul(
                                pm[:, k2 * SH : (k2 + 1) * SH],
                                kT[pt][pb : pb + ln, t * 128 : (t + 1) * 128],
                                qT[pt][pb : pb + ln, hsl],
                                start=True,
                                stop=True,
                                tile_position=tp,
                            )
                        nc.scalar.activation(
                            E[:, t2 * 2 * SH : (t2 + 1) * 2 * SH], pm[:], AF.Exp, scale=scale
                        )
                    # ctx_unnorm^T and denominator via aug ones column
                    pu = ps.tile([128, SH], F32, name=f"pu{sg}", tag="m2", bufs=2)
                    a0 = AUG_OFF[sg]
                    for t in range(NTC):
                        nc.tensor.matmul(
                            pu[0 : d + 1, :],
                            vaug[t][:, a0 : a0 + d + 1],
                            E[:, t * SH : (t + 1) * SH],
                            start=(t == 0),
                            stop=(t == NTC - 1),
                        )
                    u = spool.tile([96, SH], F32, name=f"u{sg}", tag="u")
                    nc.vector.tensor_copy(u[0 : d + 1, :], pu[0 : d + 1, :])
                    den = spool.tile([1, SH], F32, name=f"den{sg}", tag="den", bufs=1)
                    nc.gpsimd.dma_start(out=den[:], in_=u[d : d + 1, :])
                    rec = spool.tile([1, SH], F32, name=f"rec{sg}", tag="rec", bufs=1)
                    scr = spool.tile([1, SH], F32, name=f"scr{sg}", tag="scr", bufs=1)
                    nc.vector.reciprocal_approx_accurate(rec[:], den[:], scratch=scr[:])
                    recr = spool.tile([1, SH], F32R, name=f"recr{sg}", tag="recr", bufs=1)
                    nc.vector.tensor_copy(recr[:], rec[:])
                    pb_ps = ps.tile([128, SH], F32, name=f"pb{sg}", tag="x", bufs=2)
                    nc.tensor.matmul(pb_ps[0:d, :], ones_c[:, 0:d], recr[:], start=True, stop=True)
                    bc = spool.tile([96, SH], F32, name=f"bc{sg}", tag="bc")
                    nc.vector.tensor_copy(bc[0:d, :], pb_ps[0:d, :])
                    nc.vector.tensor_mul(
                        cxT[pt][pb : pb + d, hsl], u[0:d, :], bc[0:d, :]
                    )

                # ---- output projection for this half ----
                for hc, (h0, hw) in enumerate(HCH):
                    pos = []
                    for g, plist in enumerate(WO_GROUPS):
                        po = ps.tile([128, SH], F32, name=f"po{hc}{g}", tag="m2", bufs=2)
                        for j, (_, off, ln, pt, pb) in enumerate(plist):
                            nc.tensor.matmul(
                                po[0:hw, :],
                                Wo_sb[pt][pb : pb + ln, h0 : h0 + hw],
                                cxT[pt][pb : pb + ln, hsl],
                                start=(j == 0),
                                stop=(j == len(plist) - 1),
                                tile_position=(96, 0) if pb == 96 else None,
                            )
                        pos.append(po)
                    osb = spool.tile([128, SH], F32, name=f"osb{hc}", tag="osb", bufs=2)
                    nc.vector.tensor_scalar_add(osb[0:hw, :], pos[0][0:hw, :], bo_sb[hc][:])
                    for po in pos[1:]:
                        nc.vector.tensor_add(osb[0:hw, :], osb[0:hw, :], po[0:hw, :])
                    nc.sync.dma_start(out=outT[b, h0 : h0 + hw, hsl], in_=osb[0:hw, :])

    nc.compile()
    return nc


def _pad_w(W):
    f32 = np.float32
    return np.ascontiguousarray(
        np.concatenate([W.astype(f32, copy=False), np.zeros((HID, 1), f32)], axis=1)
    )


def _prep_core_inputs(hidden_states, Wq, bq, Wk, bk, Wv, bv, Wo, bo):
    """Host-side layout prep (transpose/reorder/pad only, no math)."""
    f32 = np.float32
    hs = np.ascontiguousarray(hidden_states.astype(f32, copy=False))
    Wo_p = np.zeros((NPT, 128, HID2), dtype=f32)
    for seg, off, ln, pt, pb in PIECES:
        g0 = BOUNDS[seg] + off
        Wo_p[pt, pb : pb + ln, :HID] = Wo[g0 : g0 + ln, :]
    bvb = np.broadcast_to(bv.astype(f32, copy=False), (128, HID)).copy()
    shared = {
        "Wq": _pad_w(Wq),
        "Wk": _pad_w(Wk),
        "Wv": _pad_w(Wv),
        "Wop": Wo_p,
        "bq": np.ascontiguousarray(bq.astype(f32, copy=False).reshape(HID, 1)),
        "bk": np.ascontiguousarray(bk.astype(f32, copy=False).reshape(HID, 1)),
        "bo": np.ascontiguousarray(bo.astype(f32, copy=False).reshape(HID, 1)),
        "bvb": bvb,
    }
    in_maps = []
    for c in range(N_CORES):
        shard = hs[c * BPC : (c + 1) * BPC]
        m = dict(shared)
        m["hsT"] = np.ascontiguousarray(shard.transpose(0, 2, 1))
        in_maps.append(m)
    return in_maps


LAST_RESULTS = None


def kernel(hidden_states, Wq, bq, Wk, bk, Wv, bv, Wo, bo):
    global LAST_RESULTS
    if "nc" not in _CACHE:
        _CACHE["nc"] = _build()
    nc = _CACHE["nc"]
    in_maps = _prep_core_inputs(hidden_states, Wq, bq, Wk, bk, Wv, bv, Wo, bo)
    kwargs = {}
    if os.environ.get("KERNEL_TRACE") == "1":
        kwargs["trace"] = True
        td = os.environ.get("KERNEL_TRACE_DIR")
        if td:
            kwargs["tmpdir"] = td
    res = run_bass_kernel_spmd(nc, in_maps, core_ids=list(range(N_CORES)), **kwargs)
    LAST_RESULTS = res
    out = np.empty((B, S, HID), dtype=np.float32)
    for c in range(N_CORES):
        out[c * BPC : (c + 1) * BPC] = res.results[c]["outT"].transpose(0, 2, 1)
    return out


# revision 17
# speedup vs baseline: 1.5962x; 1.0562x over previous
"""Trainium2 Bass kernel for segmented ("sparse") attention.

Reference math (per batch of 16, S=1024, H=441):
  q = hs @ Wq + bq ; k = hs @ Wk + bk ; v = hs @ Wv + bv
  for each of 9 column segments [s,e): softmax(q_i k_i^T / sqrt(d_i)) @ v_i
  out = concat(ctx_i) @ Wo + bo

Sharding: pure data parallel over batch, 2 batches per NeuronCore x 8 cores.

Device-side strategy (per batch):
  - hs pre-transposed on host: hsT [441, 1024] (h on partitions)
  - qT,kT [441, 1024] projections stored in a packed layout of 5 tiles where
    each segment's rows sit at a 32-aligned partition base
  - scoresT[t, s] = kT_seg[:, tcol].T @ qT_seg -> PSUM [128t, 512s]
  - E = exp(scoresT / sqrt(d)) on the scalar engine, PSUM -> SBUF
  - v in natural layout [s, 441] with a ones column appended per segment
    (aug layout): one accumulated matmul over the 8 t-chunks yields both
    ctx_unnorm^T and the softmax denominator
  - normalize: denom row DMA-gathered, reciprocal on DVE, broadcast across
    partitions with a K=1 ones matmul, multiplied into packed ctxT
  - outT[ho, s] = Wo_packed.T @ ctxT (+bo), accumulated per base-partition
    group (fp32r groups must not mix lhsT base partitions) and summed on DVE
  - host transposes outT back to [S, 441]

All matmuls are float32r (TF32-like). fp32r oddities honored: moving/dst
free sizes even, out base partition 0, uniform base per accumulation group.
"""

import os
import math
import numpy as np
from contextlib import ExitStack

import concourse.bacc as bacc
import concourse.tile as tile
import concourse.mybir as mybir
from concourse.bass_utils import run_bass_kernel_spmd

F32 = mybir.dt.float32
F32R = mybir.dt.float32r
BF16 = mybir.dt.bfloat16
AF = mybir.ActivationFunctionType

HID = 441
HID2 = HID + 1  # even-padded weight width
S = 1024
B = 16
N_CORES = 8
BPC = B // N_CORES
BOUNDS = [0, 7, 21, 49, 105, 161, 217, 273, 357, 441]
NSEG = 9
DSEG = [BOUNDS[i + 1] - BOUNDS[i] for i in range(NSEG)]
NHC = 4
HCH = [(i * 128, min(128, HID - i * 128)) for i in range(NHC)]
NTC = 8
SH = 512
NPT = 5

# Packed row layout: (seg, off_within_seg, length, pack_tile, pack_base)
# One piece per segment; bases 32-aligned; base 96 passed as explicit
# tile_position (auto-derive only allows 0/32/64).
PIECES = [
    (0, 0, 7, 4, 0),
    (1, 0, 14, 1, 96),
    (2, 0, 28, 0, 96),
    (3, 0, 56, 2, 0),
    (4, 0, 56, 3, 0),
    (5, 0, 56, 2, 64),
    (6, 0, 56, 3, 64),
    (7, 0, 84, 0, 0),
    (8, 0, 84, 1, 0),
]
SEG_PIECE = {p[0]: p for p in PIECES}

# output projection: one PSUM accumulation group per lhsT base partition
WO_GROUPS = [[p for p in PIECES if p[4] == b] for b in (0, 64, 96)]

# v augmented layout: per segment [v columns (d), ones column]
AUG_OFF = [BOUNDS[i] + i for i in range(NSEG)]
AUG_W = HID + NSEG  # 450


def _repack_jobs():
    jobs = []
    for seg, off, length, pt, pb in PIECES:
        g0 = BOUNDS[seg] + off
        done = 0
        while done < length:
            g = g0 + done
            ac = g // 128
            take = min(length - done, (ac + 1) * 128 - g)
            jobs.append((ac, g - ac * 128, pt, pb + done, take))
            done += take
    return jobs


REPACK = _repack_jobs()

_CACHE = {}


def _build():
    nc = bacc.Bacc("TRN2", target_bir_lowering=False, debug=False)

    hsT = nc.dram_tensor("hsT", [BPC, HID, S], F32R, kind="ExternalInput").ap()
    Wq_d = nc.dram_tensor("Wq", [HID, HID2], F32R, kind="ExternalInput").ap()
    Wk_d = nc.dram_tensor("Wk", [HID, HID2], F32R, kind="ExternalInput").ap()
    Wv_d = nc.dram_tensor("Wv", [HID, HID2], F32R, kind="ExternalInput").ap()
    Wo_d = nc.dram_tensor("Wop", [NPT, 128, HID2], F32R, kind="ExternalInput").ap()
    bq_d = nc.dram_tensor("bq", [HID, 1], F32, kind="ExternalInput").ap()
    bk_d = nc.dram_tensor("bk", [HID, 1], F32, kind="ExternalInput").ap()
    bo_d = nc.dram_tensor("bo", [HID, 1], F32, kind="ExternalInput").ap()
    bvb_d = nc.dram_tensor("bvb", [128, HID], F32, kind="ExternalInput").ap()
    ind9_d = nc.dram_tensor("ind9", [NSEG, NSEG * 128], F32R, kind="ExternalInput").ap()
    outT = nc.dram_tensor("outT", [BPC, HID, S], F32, kind="ExternalOutput").ap()

    with tile.TileContext(nc) as tc, ExitStack() as ctx, nc.allow_low_precision(
        reason="float32r rounding for matmul inputs"
    ):
        cpool = ctx.enter_context(tc.tile_pool(name="c", bufs=1))
        hpool = ctx.enter_context(tc.tile_pool(name="h", bufs=1))
        apool = ctx.enter_context(tc.tile_pool(name="a", bufs=1))
        ppool = ctx.enter_context(tc.tile_pool(name="p", bufs=1))
        epool = ctx.enter_context(tc.tile_pool(name="e", bufs=2))
        vpool = ctx.enter_context(tc.tile_pool(name="v", bufs=9))
        spool = ctx.enter_context(tc.tile_pool(name="s", bufs=2))
        ps = ctx.enter_context(tc.tile_pool(name="ps", bufs=1, space="PSUM"))

        # ---- constants ----
        Wq_sb, Wk_sb, Wv_sb = [], [], []
        for hc, (h0, hw) in enumerate(HCH):
            for lst, src, nm in ((Wq_sb, Wq_d, "wq"), (Wk_sb, Wk_d, "wk"), (Wv_sb, Wv_d, "wv")):
                t = cpool.tile([hw, HID2], F32R, name=f"{nm}{hc}", tag=f"{nm}{hc}")
                nc.sync.dma_start(out=t, in_=src[h0 : h0 + hw, :])
                lst.append(t)
        Wo_sb = []
        for i in range(NPT):
            t = cpool.tile([128, HID2], F32R, name=f"wo{i}", tag=f"wo{i}")
            nc.sync.dma_start(out=t, in_=Wo_d[i])
            Wo_sb.append(t)
        bq_sb, bk_sb, bo_sb = [], [], []
        for hc, (h0, hw) in enumerate(HCH):
            for lst, src, nm in ((bq_sb, bq_d, "bq"), (bk_sb, bk_d, "bk"), (bo_sb, bo_d, "bo")):
                t = cpool.tile([hw, 1], F32, name=f"{nm}{hc}", tag=f"{nm}{hc}")
                nc.sync.dma_start(out=t, in_=src[h0 : h0 + hw, :])
                lst.append(t)
        bvb = cpool.tile([128, HID], F32, name="bvb", tag="bvb")
        nc.sync.dma_start(out=bvb, in_=bvb_d)
        ind9 = cpool.tile([NSEG, NSEG * 128], F32R, name="ind9", tag="ind9")
        nc.sync.dma_start(out=ind9, in_=ind9_d)

        for b in range(BPC):
            # ---- load hsT ----
            hs = []
            for hc, (h0, hw) in enumerate(HCH):
                t = hpool.tile([hw, S], F32R, name=f"hs{hc}", tag=f"hs{hc}", bufs=1)
                nc.sync.dma_start(out=t, in_=hsT[b, h0 : h0 + hw, :])
                hs.append(t)

            # ---- q/k projections into aligned chunks, DMA-repacked ----
            packs = {}
            for nm, W_sb, b_sb in (("q", Wq_sb, bq_sb), ("k", Wk_sb, bk_sb)):
                al = []
                for mc, (m0, mw) in enumerate(HCH):
                    qa = apool.tile([mw, S], BF16, name=f"al{nm}{mc}", tag=f"al{mc}", bufs=1)
                    for half in range(2):
                        pa = ps.tile([128, SH], F32, name=f"pp{nm}{mc}{half}", tag="x", bufs=2)
                        for hc, (h0, hw) in enumerate(HCH):
                            nc.tensor.matmul(
                                pa[0:mw, :],
                                W_sb[hc][:, m0 : m0 + mw],
                                hs[hc][:, half * SH : (half + 1) * SH],
                                start=(hc == 0),
                                stop=(hc == NHC - 1),
                            )
                        nc.vector.tensor_scalar_add(
                            qa[:, half * SH : (half + 1) * SH], pa[0:mw, :], b_sb[mc][:]
                        )
                    al.append(qa)
                pk = []
                for i in range(NPT):
                    t = ppool.tile([128, S], BF16, name=f"{nm}T{i}", tag=f"{nm}T{i}")
                    pk.append(t)
                for ac, r0, pt, pb, ln in REPACK:
                    nc.gpsimd.dma_start(out=pk[pt][pb : pb + ln, :], in_=al[ac][r0 : r0 + ln, :])
                packs[nm] = pk
            qT, kT = packs["q"], packs["k"]

            # ---- v projection (natural layout) + aug ones columns ----
            vaug = []
            for sc in range(NTC):
                pv = ps.tile([128, HID2], F32, name=f"pv{sc}", tag="x", bufs=2)
                for hc, (h0, hw) in enumerate(HCH):
                    nc.tensor.matmul(
                        pv[:],
                        hs[hc][:, sc * 128 : (sc + 1) * 128],
                        Wv_sb[hc][:],
                        start=(hc == 0),
                        stop=(hc == NHC - 1),
                    )
                vt = spool.tile([128, HID], BF16, name=f"vt{sc}", tag="vt")
                nc.vector.tensor_add(vt[:], pv[:, 0:HID], bvb[:])
                va = vpool.tile([128, AUG_W], BF16, name=f"va{sc}", tag="va")
                for sg in range(NSEG):
                    s0, s1 = BOUNDS[sg], BOUNDS[sg + 1]
                    a0 = AUG_OFF[sg]
                    nc.gpsimd.dma_start(out=va[:, a0 : a0 + (s1 - s0)], in_=vt[:, s0:s1])
                    nc.vector.memset(va[:, a0 + (s1 - s0) : a0 + (s1 - s0) + 1], 1.0)
                vaug.append(va)

            # ---- attention ----
            cxT = [ppool.tile([128, S], F32R, name=f"cxT{i}", tag=f"cxT{i}") for i in range(NPT)]
            for half in range(2):
                hsl = slice(half * SH, (half + 1) * SH)
                us = [
                    spool.tile([96, SH], F32, name=f"u{sg}", tag=f"u{sg}", bufs=2)
                    for sg in range(NSEG)
                ]
                den9 = spool.tile([NSEG, SH], F32, name="den9", tag="den9", bufs=2)
                for sg in range(NSEG):
                    d = DSEG[sg]
                    scale = 1.0 / math.sqrt(d)
                    _, off, ln, pt, pb = SEG_PIECE[sg]
                    tp = (96, 0) if pb == 96 else None
                    E = epool.tile([128, NTC * SH], BF16, name=f"E{sg}", tag="E")
                    for t2 in range(NTC // 2):
                        pm = ps.tile([128, 2 * SH], F32, name=f"pm{sg}{t2}", tag="m1", bufs=2)
                        for k2 in range(2):
                            t = 2 * t2 + k2
                            nc.tensor.matmul(
                                pm[:, k2 * SH : (k2 + 1) * SH],
                                kT[pt][pb : pb + ln, t * 128 : (t + 1) * 128],
                                qT[pt][pb : pb + ln, hsl],
                                start=True,
                                stop=True,
                                tile_position=tp,
                            )
                        nc.scalar.activation(
                            E[:, t2 * 2 * SH : (t2 + 1) * 2 * SH], pm[:], AF.Exp, scale=scale
                        )
                    # ctx_unnorm^T and denominator via aug ones column
                    pu = ps.tile([128, SH], F32, name=f"pu{sg}", tag="m2", bufs=2)
                    a0 = AUG_OFF[sg]
                    for t in range(NTC):
                        nc.tensor.matmul(
                            pu[0 : d + 1, :],
                            vaug[t][:, a0 : a0 + d + 1],
                            E[:, t * SH : (t + 1) * SH],
                            start=(t == 0),
                            stop=(t == NTC - 1),
                        )
                    u = us[sg]
                    nc.vector.tensor_copy(u[0 : d + 1, :], pu[0 : d + 1, :])
                    nc.gpsimd.dma_start(out=den9[sg : sg + 1, :], in_=u[d : d + 1, :])
                # batched reciprocal of all 9 denominators, then per-seg
                # partition-broadcast via indicator matmul
                rec9 = spool.tile([NSEG, SH], F32, name="rec9", tag="rec9", bufs=2)
                scr9 = spool.tile([NSEG, SH], F32, name="scr9", tag="scr9", bufs=2)
                nc.vector.reciprocal_approx_accurate(rec9[:], den9[:], scratch=scr9[:])
                rec9r = spool.tile([NSEG, SH], F32R, name="rec9r", tag="rec9r", bufs=2)
                nc.vector.tensor_copy(rec9r[:], rec9[:])
                for sg in range(NSEG):
                    d = DSEG[sg]
                    _, off, ln, pt, pb = SEG_PIECE[sg]
                    pb_ps = ps.tile([128, SH], F32, name=f"pb{sg}", tag="x", bufs=2)
                    nc.tensor.matmul(
                        pb_ps[0:d, :], ind9[:, sg * 128 : sg * 128 + d], rec9r[:],
                        start=True, stop=True,
                    )
                    bc = spool.tile([96, SH], F32, name=f"bc{sg}", tag="bc")
                    nc.vector.tensor_copy(bc[0:d, :], pb_ps[0:d, :])
                    nc.vector.tensor_mul(
                        cxT[pt][pb : pb + d, hsl], us[sg][0:d, :], bc[0:d, :]
                    )

                # ---- output projection for this half ----
                for hc, (h0, hw) in enumerate(HCH):
                    pos = []
                    for g, plist in enumerate(WO_GROUPS):
                        po = ps.tile([128, SH], F32, name=f"po{hc}{g}", tag="m2", bufs=2)
                        for j, (_, off, ln, pt, pb) in enumerate(plist):
                            nc.tensor.matmul(
                                po[0:hw, :],
                                Wo_sb[pt][pb : pb + ln, h0 : h0 + hw],
                                cxT[pt][pb : pb + ln, hsl],
                                start=(j == 0),
                                stop=(j == len(plist) - 1),
                                tile_position=(96, 0) if pb == 96 else None,
                            )
                        pos.append(po)
                    osb = spool.tile([128, SH], F32, name=f"osb{hc}", tag="osb", bufs=2)
                    nc.vector.tensor_scalar_add(osb[0:hw, :], pos[0][0:hw, :], bo_sb[hc][:])
                    for po in pos[1:]:
                        nc.vector.tensor_add(osb[0:hw, :], osb[0:hw, :], po[0:hw, :])
                    nc.sync.dma_start(out=outT[b, h0 : h0 + hw, hsl], in_=osb[0:hw, :])

    nc.compile()
    return nc


def _pad_w(W):
    f32 = np.float32
    return np.ascontiguousarray(
        np.concatenate([W.astype(f32, copy=False), np.zeros((HID, 1), f32)], axis=1)
    )


def _prep_core_inputs(hidden_states, Wq, bq, Wk, bk, Wv, bv, Wo, bo):
    """Host-side layout prep (transpose/reorder/pad only, no math)."""
    f32 = np.float32
    hs = np.ascontiguousarray(hidden_states.astype(f32, copy=False))
    Wo_p = np.zeros((NPT, 128, HID2), dtype=f32)
    for seg, off, ln, pt, pb in PIECES:
        g0 = BOUNDS[seg] + off
        Wo_p[pt, pb : pb + ln, :HID] = Wo[g0 : g0 + ln, :]
    bvb = np.broadcast_to(bv.astype(f32, copy=False), (128, HID)).copy()
    ind9 = np.zeros((NSEG, NSEG * 128), dtype=f32)
    for sg in range(NSEG):
        ind9[sg, sg * 128 : sg * 128 + DSEG[sg]] = 1.0
    shared = {
        "Wq": _pad_w(Wq),
        "Wk": _pad_w(Wk),
        "Wv": _pad_w(Wv),
        "Wop": Wo_p,
        "bq": np.ascontiguousarray(bq.astype(f32, copy=False).reshape(HID, 1)),
        "bk": np.ascontiguousarray(bk.astype(f32, copy=False).reshape(HID, 1)),
        "bo": np.ascontiguousarray(bo.astype(f32, copy=False).reshape(HID, 1)),
        "bvb": bvb,
        "ind9": ind9,
    }
    in_maps = []
    for c in range(N_CORES):
        shard = hs[c * BPC : (c + 1) * BPC]
        m = dict(shared)
        m["hsT"] = np.ascontiguousarray(shard.transpose(0, 2, 1))
        in_maps.append(m)
    return in_maps


LAST_RESULTS = None


def kernel(hidden_states, Wq, bq, Wk, bk, Wv, bv, Wo, bo):
    global LAST_RESULTS
    if "nc" not in _CACHE:
        _CACHE["nc"] = _build()
    nc = _CACHE["nc"]
    in_maps = _prep_core_inputs(hidden_states, Wq, bq, Wk, bk, Wv, bv, Wo, bo)
    kwargs = {}
    if os.environ.get("KERNEL_TRACE") == "1":
        kwargs["trace"] = True
        td = os.environ.get("KERNEL_TRACE_DIR")
        if td:
            kwargs["tmpdir"] = td
    res = run_bass_kernel_spmd(nc, in_maps, core_ids=list(range(N_CORES)), **kwargs)
    LAST_RESULTS = res
    out = np.empty((B, S, HID), dtype=np.float32)
    for c in range(N_CORES):
        out[c * BPC : (c + 1) * BPC] = res.results[c]["outT"].transpose(0, 2, 1)
    return out
